# revision 20
# baseline (speedup 1.0000x reference)
"""Fused multi-head attention kernel for Trainium2, SPMD over 8 NeuronCores.

Sharding: data-parallel over batch (B=8 -> 1 batch per core). No collectives.

Per-core algorithm (all shapes per core, b fixed):
  x^T [E, L] (host-transposed), weights host-transposed/packed.
  Phase A: Q^T, K^T = Wq^T/Wk^T-stationary matmuls (f32r, full rate);
           V packed [L, H*65] bf16 with a ones column per head (col 64 of 65)
           so the PV matmul also produces the softmax denominator.
  Phase B: per (head-pair, q-half, k-chunk):
           S^T[k,q] = K Q^T  (f32r matmul, contract=A=64, auto row-tiling
           via base_partition so even/odd heads use disjoint PE row groups)
           + bias^T via PE transpose-matmul accumulation of pre-masked
           bf16 bias chunks (mask applied in natural layout with one DVE
           copy_predicated pass writing -3.4e38).
           P^T = exp(S^T) on ACT (psum->sbuf, bf16).
           values^T[a,q] (+denominator row) = [V|1]^T-stationary matmul.
           Normalize: reciprocal of denom row, PE broadcast matmul across
           partitions, DVE multiply into values^T sbuf.
  Phase C: Y = values^T-stationary @ W_out^T (f32r), DMA out.
"""

import sys

sys.path.insert(0, "/opt/trn_rl_repo")

import numpy as np
from contextlib import ExitStack

B, L, E, H, A = 8, 1024, 1024, 16, 64
SCALE = float(A) ** -0.5
NEG = float(np.finfo(np.float32).min)
HP = H // 2  # head pairs
KT = L // 128  # 8 k-chunks of 128

_cache = {}


def _build_nc():
    import concourse.bass as bass
    import concourse.bacc as bacc
    import concourse.tile as tile
    from concourse import mybir

    f32 = mybir.dt.float32
    f32r = mybir.dt.float32r
    bf16 = mybir.dt.bfloat16
    u8 = mybir.dt.uint8
    PSUM = bass.MemorySpace.PSUM
    Exp = mybir.ActivationFunctionType.Exp

    nc = bacc.Bacc(None, target_bir_lowering=False)
    xT_d = nc.dram_tensor("xT", [E, L], f32r, kind="ExternalInput")
    wq_d = nc.dram_tensor("wq", [E, E], f32r, kind="ExternalInput")
    wk_d = nc.dram_tensor("wk", [E, E], f32r, kind="ExternalInput")
    wv_d = nc.dram_tensor("wv", [E, H * 65], f32r, kind="ExternalInput")
    wo_d = nc.dram_tensor("wo", [E, E], bf16, kind="ExternalInput")
    bias_d = nc.dram_tensor("bias", [H, L, L], f32, kind="ExternalInput")
    ident_d = nc.dram_tensor("ident", [128, 128], f32, kind="ExternalInput")
    mask_d = nc.dram_tensor("mask", [H, L, L], u8, kind="ExternalInput")
    y_d = nc.dram_tensor("y", [L, E], f32, kind="ExternalOutput")

    with nc.allow_low_precision(reason="f32r feeds PE at full rate; rounding is intentional"), \
         tile.TileContext(nc) as tc, ExitStack() as top:
        pp = top.enter_context(tc.tile_pool(name="persist", bufs=8))
        cp = top.enter_context(tc.tile_pool(name="consts", bufs=1))

        qT = [pp.tile([128, L], f32r, tag="qT", name=f"qT{_}") for _ in range(8)]
        kTt = [pp.tile([128, L], f32r, tag="kT", name=f"kT{_}") for _ in range(8)]
        vs = [pp.tile([128, H * 65], bf16, tag="vs", name=f"vs{_}") for _ in range(8)]
        vT = [pp.tile([128, L], bf16, tag="vT", name=f"vT{_}") for _ in range(8)]

        ident = cp.tile([128, 128], f32, tag="ident")
        nc.gpsimd.dma_start(ident[:], ident_d[:, :])
        neg_t = cp.tile([128, 1, L], f32, tag="neg")
        nc.vector.memset(neg_t[:], NEG)
        ones1 = cp.tile([1, 64], f32, tag="ones1")
        nc.vector.memset(ones1[:], 1.0)

        # ---------------- Phase A: projections ----------------
        with tc.tile_pool(name="pa_w", bufs=3) as wp, \
             tc.tile_pool(name="pa_x", bufs=2) as xp, \
             tc.tile_pool(name="pa_ps", bufs=2, space=PSUM) as psA:
            xs4 = [xp.tile([128, 4, L], f32r, tag="xs", name=f"xs{_}") for _ in range(2)]
            for t in range(2):
                nc.gpsimd.dma_start(
                    xs4[t][:],
                    xT_d[t * 512:(t + 1) * 512, :]
                    .rearrange("(t p) e -> p t e", p=128))

            def xsl(k):
                return xs4[k // 4][:, k % 4, :]

            def proj_qk(w_d, out_tiles):
                wt4 = [wp.tile([128, 4, E], f32r, tag="wt", name=f"wt{_}") for _ in range(2)]
                for t in range(2):
                    nc.gpsimd.dma_start(
                        wt4[t][:],
                        w_d[t * 512:(t + 1) * 512, :]
                        .rearrange("(t p) e -> p t e", p=128))
                for m in range(8):
                    ps = psA.tile([128, L], f32, tag="psA")
                    for k in range(8):
                        for lh in range(2):
                            nc.tensor.matmul(
                                ps[:, lh * 512:(lh + 1) * 512],
                                wt4[k // 4][:, k % 4, m * 128:(m + 1) * 128],
                                xsl(k)[:, lh * 512:(lh + 1) * 512],
                                start=(k == 0), stop=(k == 7))
                    for lh in range(2):
                        nc.scalar.copy(out_tiles[m][:, lh * 512:(lh + 1) * 512],
                                       ps[:, lh * 512:(lh + 1) * 512])

            proj_qk(wq_d, qT)
            proj_qk(wk_d, kTt)

            # V projection: out natural [l, (h,a)+ones-slot], bf16
            wtv4 = [wp.tile([128, 4, H * 65], f32r, tag="wt", name=f"wtv{_}") for _ in range(2)]
            for t in range(2):
                nc.gpsimd.dma_start(
                    wtv4[t][:],
                    wv_d[t * 512:(t + 1) * 512, :]
                    .rearrange("(t p) e -> p t e", p=128))
            segs = [(0, 512), (512, 512), (1024, 16)]
            for lc in range(8):
                psv = psA.tile([128, H * 65], f32, tag="psA")
                for k in range(8):
                    for off, n in segs:
                        nc.tensor.matmul(
                            psv[:, off:off + n],
                            xsl(k)[:, lc * 128:(lc + 1) * 128],
                            wtv4[k // 4][:, k % 4, off:off + n],
                            start=(k == 0), stop=(k == 7))
                for off, n in segs:
                    nc.scalar.copy(vs[lc][:, off:off + n], psv[:, off:off + n])
                # ones column per head (col 64 of each 65-wide slot)
                ones_cols = vs[lc][:].rearrange("p (h c) -> p h c", c=65)[:, :, 64:65]
                nc.vector.memset(ones_cols, 1.0)

        # ---------------- Phase B: attention ----------------
        with tc.tile_pool(name="b_bm", bufs=3) as bmp, \
             tc.tile_pool(name="b_mk", bufs=3) as mkp, \
             tc.tile_pool(name="b_pt", bufs=3) as ptp, \
             tc.tile_pool(name="b_nrm", bufs=4) as nrm, \
             tc.tile_pool(name="b_st", bufs=4, space=PSUM) as stp, \
             tc.tile_pool(name="b_pv", bufs=3, space=PSUM) as pvp, \
             tc.tile_pool(name="b_bc", bufs=1, space=PSUM) as bcp:
            for hp in range(HP):
                tmpv_full = nrm.tile([64, L], bf16, tag="tmpv")
                tmpvs = [tmpv_full, tmpv_full]
                mks = []
                for i, h in enumerate((2 * hp, 2 * hp + 1)):
                    mk = mkp.tile([128, 8, L], u8, tag="mk")
                    nc.gpsimd.dma_start(
                        mk[:],
                        mask_d[h, :, :].rearrange("(qt p) k -> p qt k", p=128))
                    mks.append(mk)
                for qh in range(2):
                    bms = []
                    for i, h in enumerate((2 * hp, 2 * hp + 1)):
                        bm = bmp.tile([128, 4, L], f32, tag="bm")
                        nc.gpsimd.dma_start(
                            bm[:],
                            bias_d[h, qh * 512:(qh + 1) * 512, :]
                            .rearrange("(qt p) k -> p qt k", p=128))
                        for j in range(4):
                            nc.vector.copy_predicated(
                                bm[:, j:j + 1, :],
                                mks[i][:, qh * 4 + j:qh * 4 + j + 1, :], neg_t[:])
                        bms.append(bm)
                    pvs = [pvp.tile([65, 512], f32, tag="pv", name=f"pv{_}") for _ in range(2)]
                    for k in range(8):
                        for i, h in enumerate((2 * hp, 2 * hp + 1)):
                            hb = (h % 2) * 64
                            st = stp.tile([128, 512], f32, tag="st")
                            nc.tensor.matmul(
                                st[:],
                                kTt[hp][hb:hb + 64, k * 128:(k + 1) * 128].bitcast(f32r),
                                qT[hp][hb:hb + 64, qh * 512:(qh + 1) * 512].bitcast(f32r),
                                start=True, stop=False)
                            for j in range(4):
                                nc.tensor.matmul(
                                    st[:, j * 128:(j + 1) * 128],
                                    bms[i][:, j, k * 128:(k + 1) * 128],
                                    ident[:],
                                    is_transpose=True,
                                    start=False, stop=(j == 3),
                                    skip_group_check=True)
                            pt = ptp.tile([128, 512], bf16, tag="pt")
                            nc.scalar.activation(pt[:], st[:], Exp)
                            nc.tensor.matmul(
                                pvs[i][:],
                                vs[k][:, h * 65:(h + 1) * 65],
                                pt[:],
                                start=(k == 0), stop=(k == 7))
                    for i, h in enumerate((2 * hp, 2 * hp + 1)):
                        stage = nrm.tile([1, 512], f32, tag="stage")
                        nc.vector.reciprocal(stage[:], pvs[i][64:65, :])
                        psb = bcp.tile([64, 512], f32, tag="psb")
                        nc.tensor.matmul(psb[:], ones1[:], stage[:],
                                         start=True, stop=True)
                        recipb = nrm.tile([64, 512], f32, tag="recipb")
                        nc.scalar.copy(recipb[:], psb[:])
                        dst = vT[hp][hb0(h):hb0(h) + 64, qh * 512:(qh + 1) * 512]
                        if h % 2 == 0:
                            nc.vector.tensor_mul(dst, pvs[i][0:64, :], recipb[:])
                        else:
                            tmpv = tmpvs[qh]
                            nc.vector.tensor_mul(tmpv[:, qh * 512:(qh + 1) * 512],
                                                 pvs[i][0:64, :], recipb[:])
                            if qh == 1:
                                nc.gpsimd.dma_start(
                                    vT[hp][64:128, :], tmpv[:, :])

        # ---------------- Phase C: output projection ----------------
        with tc.tile_pool(name="c_wo", bufs=8) as wop, \
             tc.tile_pool(name="c_y", bufs=2) as yp, \
             tc.tile_pool(name="c_ps", bufs=2, space=PSUM) as psC:
            wot = [wop.tile([128, E], bf16, tag="wo", name=f"wo{_}") for _ in range(8)]
            for t in range(8):
                nc.gpsimd.dma_start(wot[t][:], wo_d[t * 128:(t + 1) * 128, :])
            for lc2 in range(4):
                y = yp.tile([128, 2, E], f32, tag="y")
                for half in range(2):
                    lc = lc2 * 2 + half
                    psy = psC.tile([128, E], f32, tag="psy")
                    for ec in range(8):
                        for eh in range(2):
                            nc.tensor.matmul(
                                psy[:, eh * 512:(eh + 1) * 512],
                                vT[ec][:, lc * 128:(lc + 1) * 128],
                                wot[ec][:, eh * 512:(eh + 1) * 512],
                                start=(ec == 0), stop=(ec == 7))
                    for eh in range(2):
                        nc.scalar.copy(y[:, half, eh * 512:(eh + 1) * 512],
                                       psy[:, eh * 512:(eh + 1) * 512])
                nc.gpsimd.dma_start(
                    y_d[lc2 * 256:(lc2 + 1) * 256, :]
                    .rearrange("(t p) e -> p t e", p=128), y[:])

    nc.finalize()
    return nc


def hb0(h):
    return (h % 2) * 64


def _prep_host(inputs):
    emb = np.asarray(inputs["embeddings"], np.float32)
    mask = np.asarray(inputs["attn_mask"])
    bias = np.asarray(inputs["attn_bias"], np.float32)
    Wqkv = np.asarray(inputs["W_qkv"], np.float32)
    Wout = np.asarray(inputs["W_out"], np.float32)

    Wr = Wqkv.reshape(H, 3 * A, E)
    WqT = np.ascontiguousarray((Wr[:, 0:A, :].reshape(E, E) * SCALE).T)
    WkT = np.ascontiguousarray(Wr[:, A:2 * A, :].reshape(E, E).T)
    Wv_T = Wr[:, 2 * A:3 * A, :].reshape(E, E).T  # [e, (h,a)]
    WvT = np.zeros((E, H * 65), np.float32)
    for h in range(H):
        WvT[:, h * 65:h * 65 + 64] = Wv_T[:, h * 64:(h + 1) * 64]
    WvT = np.ascontiguousarray(WvT)
    import ml_dtypes
    WoT = np.ascontiguousarray(Wout.T.astype(ml_dtypes.bfloat16))

    if mask.dtype == np.bool_:
        mask_u8 = mask.view(np.uint8)
    else:
        mask_u8 = (mask != 0).astype(np.uint8)

    ident_np = np.eye(128, dtype=np.float32)
    in_maps = []
    for b in range(B):
        in_maps.append({
            "xT": np.ascontiguousarray(emb[b].T),
            "wq": WqT, "wk": WkT, "wv": WvT, "wo": WoT,
            "bias": np.ascontiguousarray(bias[b]),
            "mask": np.ascontiguousarray(mask_u8[b]),
            "ident": ident_np,
        })
    return in_maps


def _run(inputs, trace=False):
    from concourse.bass_utils import run_bass_kernel_spmd

    if "nc" not in _cache:
        _cache["nc"] = _build_nc()
    nc = _cache["nc"]
    in_maps = _prep_host(inputs)
    res = run_bass_kernel_spmd(nc, in_maps, core_ids=list(range(8)), trace=trace)
    out = np.stack([np.asarray(res.results[c]["y"], np.float32) for c in range(B)], axis=0)
    return out, res


def kernel(**inputs) -> np.ndarray:
    out, _ = _run(inputs, trace=False)
    return out


def kernel_traced(**inputs):
    return _run(inputs, trace=True)
